# revision 17
# baseline (speedup 1.0000x reference)
"""Gemma GQA self-attention prefill on 8 TRN2 NeuronCores.

Sharding: core c owns KV head c and its two query heads {c, c+8}
(the reference maps q head H to kv head H % 8).  Each core computes
qT/kT/v projections for its slice directly in transposed layouts,
runs causal attention in the S^T formulation (keys on partitions, so
softmax sums come from a ones-matmul and the AV matmul needs no
transposes), normalizes, then an AllToAll switches to token sharding
and each core applies the full (row-permuted) W_o to its 256-token
slice.  Host assembles the 8 token slices.

All matmuls run in bf16 (fp32 accumulation in PSUM).
"""

import contextlib
import ctypes
import os
import sys
import types

import numpy as np


def _install_ntff_hook():
    """bass_utils under axon imports antenv.axon_hooks, which this image's
    antenv stub lacks.  Recreate the hook via ctypes on libaxon_pjrt."""
    if "antenv.axon_hooks" in sys.modules:
        return
    hook = None
    so_path = "/opt/axon/libaxon_pjrt.so"
    try:
        lib = ctypes.CDLL(so_path)
        if hasattr(lib, "axon_start_nrt_profile"):
            lib.axon_start_nrt_profile.argtypes = [
                ctypes.POINTER(ctypes.c_int64),
                ctypes.c_size_t,
            ]
            lib.axon_start_nrt_profile.restype = ctypes.c_int64
            lib.axon_stop_nrt_profile.argtypes = [ctypes.c_char_p]
            lib.axon_stop_nrt_profile.restype = ctypes.c_int64

            @contextlib.contextmanager
            def hook(output_dir, device_ids):
                import jax

                jax.devices()
                if device_ids:
                    ids = (ctypes.c_int64 * len(device_ids))(*device_ids)
                    rc = lib.axon_start_nrt_profile(ids, len(device_ids))
                else:
                    rc = lib.axon_start_nrt_profile(None, 0)
                if rc != 0:
                    raise RuntimeError(f"axon_start_nrt_profile rc={rc}")
                try:
                    yield
                finally:
                    n = lib.axon_stop_nrt_profile(str(output_dir).encode())
                    print(f"profile: {n} file(s) in {output_dir}", file=sys.stderr)

    except OSError:
        hook = None
    mod = types.ModuleType("antenv.axon_hooks")
    mod.get_axon_ntff_profile_hook = lambda: hook
    mod.set_axon_ntff_profile_hook = lambda h: None
    sys.modules["antenv.axon_hooks"] = mod


_install_ntff_hook()

import ml_dtypes  # noqa: E402
import concourse.bass as bass  # noqa: E402
import concourse.mybir as mybir  # noqa: E402
from concourse.bass_utils import run_bass_kernel_spmd  # noqa: E402

BF16 = mybir.dt.bfloat16
F32 = mybir.dt.float32

N_CORES = 8
T = 2048          # sequence length
HID = 3072        # hidden
KC = 24           # hidden chunks of 128
D = 256           # head dim
NH = 16           # q heads
NKV = 8           # kv heads
TS = T // N_CORES  # 256 tokens per core after A2A

# attention tile lists: per local head h (0,1), t-tile j (4 of 512),
# u-tile i (16 of 128); causal keeps i <= 4j+3
TILES = [
    (h, j, i) for h in range(2) for j in range(4) for i in range(4 * j + 4)
]
N_TILES = len(TILES)  # 80
GROUP_OF = {}
G_FIRST = {}
G_LAST = {}
for idx, (h, j, i) in enumerate(TILES):
    g = 4 * h + j
    GROUP_OF[idx] = g
    G_FIRST.setdefault(g, idx)
    G_LAST[g] = idx
DIAG_COUNT = []  # number of diagonal (masked) tiles with index <= idx
_dc = 0
for idx, (h, j, i) in enumerate(TILES):
    if i >= 4 * j:
        _dc += 1
    DIAG_COUNT.append(_dc)


def build_program():
    nc = bass.Bass(trn_type="TRN2", num_devices=N_CORES)

    xt = nc.dram_tensor("xt", [HID, T], BF16, kind="ExternalInput")
    wqk = nc.dram_tensor("wqk", [HID, 768], BF16, kind="ExternalInput")
    wv = nc.dram_tensor("wv", [HID, 256], BF16, kind="ExternalInput")
    cosT = nc.dram_tensor("cosT", [128, T], F32, kind="ExternalInput")
    sinT = nc.dram_tensor("sinT", [128, T], F32, kind="ExternalInput")
    masks = nc.dram_tensor("masks", [128, 4 * 512], BF16, kind="ExternalInput")
    wo = nc.dram_tensor("wo", [4096, HID], BF16, kind="ExternalInput")
    out = nc.dram_tensor("out", [TS, HID], F32, kind="ExternalOutput")

    a2a_in = nc.dram_tensor("a2a_in", [8, 512, 256], BF16)
    a2a_out = nc.dram_tensor("a2a_out", [8, 512, 256], BF16)

    ctx = contextlib.ExitStack()
    with ctx:
        # ---- SBUF ----
        xt_sb = ctx.enter_context(nc.sbuf_tensor("xt_sb", [128, 2, KC, 512], BF16))
        wqk_sb = ctx.enter_context(nc.sbuf_tensor("wqk_sb", [128, KC, 768], BF16))
        wv_sb = ctx.enter_context(nc.sbuf_tensor("wv_sb", [128, KC, 256], BF16))
        cos_sb = ctx.enter_context(nc.sbuf_tensor("cos_sb", [128, T], F32))
        sin_sb = ctx.enter_context(nc.sbuf_tensor("sin_sb", [128, T], F32))
        mask_sb = ctx.enter_context(nc.sbuf_tensor("mask_sb", [128, 4, 512], BF16))
        qk_sb = ctx.enter_context(nc.sbuf_tensor("qk_sb", [128, 6, T], BF16))
        v_sb = ctx.enter_context(nc.sbuf_tensor("v_sb", [128, 16, 256], BF16))
        pt_sb = ctx.enter_context(nc.sbuf_tensor("pt_sb", [128, 4, 512], BF16))
        ot_sb = ctx.enter_context(nc.sbuf_tensor("ot_sb", [128, 2, 512], BF16))
        rb_sb = ctx.enter_context(nc.sbuf_tensor("rb_sb", [128, 512], F32))
        rcrow = ctx.enter_context(nc.sbuf_tensor("rcrow", [1, 512], F32))
        tmpA = ctx.enter_context(nc.sbuf_tensor("tmpA", [128, 512], F32))
        tmpB = ctx.enter_context(nc.sbuf_tensor("tmpB", [128, 512], F32))
        ones_sb = ctx.enter_context(nc.sbuf_tensor("ones_sb", [128, 1], BF16))
        ones_row = ctx.enter_context(nc.sbuf_tensor("ones_row", [1, 128], F32))
        r_sb = ctx.enter_context(nc.sbuf_tensor("r_sb", [128, 32, 256], BF16))
        wo_sb = ctx.enter_context(nc.sbuf_tensor("wo_sb", [128, 2, 8, 512], BF16))
        outst = ctx.enter_context(nc.sbuf_tensor("outst", [128, 2, 512], F32))

        # ---- PSUM (7 full banks) ----
        A = [
            ctx.enter_context(nc.psum_tensor(f"psA{i}", [128, 512], F32))
            for i in range(4)
        ]
        B = [
            ctx.enter_context(nc.psum_tensor(f"psB{i}", [128, 512], F32))
            for i in range(2)
        ]
        CS = ctx.enter_context(nc.psum_tensor("psCS", [128, 512], F32))
        BC = ctx.enter_context(nc.psum_tensor("psBC", [128, 512], F32))

        # ---- semaphores ----
        sems = {}
        for name in (
            "s_init", "s_xt0", "s_xt1", "s_xt2", "s_xt3", "s_woA", "s_woB",
            "s_r", "s_a2a", "s_out", "s_pq", "s_pv", "s_pqd", "s_vcp",
            "s_stp", "s_exp", "s_mask", "s_ptc", "s_av", "s_rc", "s_bc",
            "s_norm", "s_cc", "s_po", "s_oc", "s_wod", "s_misc", "s_dve",
        ):
            sems[name] = ctx.enter_context(nc.semaphore(name))
        S = types.SimpleNamespace(**sems)

        ld_count = [0]  # number of s_ld DMAs issued (each incs by 16)
        mile = {}

        with nc.Block() as block:

            # ---------------- SYNC: all DMA ----------------
            @block.sync
            def _(sync):
                def ld(sem, out_ap, in_ap):
                    sync.dma_start(out_ap, in_ap).then_inc(sem, 16)

                # initial loads
                ld(S.s_init, wqk_sb[:, :, :], wqk[:, :].rearrange("(c p) m -> p c m", p=128))
                ld(S.s_init, wv_sb[:, :, :], wv[:, :].rearrange("(c p) m -> p c m", p=128))
                ld(S.s_init, cos_sb[:, :], cosT[:, :])
                ld(S.s_init, sin_sb[:, :], sinT[:, :])
                ld(S.s_init, mask_sb[:, :, :], masks[:, :].rearrange("p (m f) -> p m f", m=4))

                # xt batches (t-tile b covers t in [512b, 512b+512))
                xt_sems = [S.s_xt0, S.s_xt1, S.s_xt2, S.s_xt3]

                def xt_batch(b):
                    for s in range(8):
                        src = xt[384 * s:384 * s + 384, 512 * b:512 * b + 512]
                        ld(
                            xt_sems[b],
                            xt_sb[:, b % 2, 3 * s:3 * s + 3, :],
                            src.rearrange("(c p) t -> p c t", p=128),
                        )

                xt_batch(0)
                xt_batch(1)
                sync.wait_ge(S.s_pq, 6)
                sync.wait_ge(S.s_pv, 4)
                xt_batch(2)
                sync.wait_ge(S.s_pq, 12)
                sync.wait_ge(S.s_pv, 8)
                xt_batch(3)

                # wo batch w = 4*n + kb -> chunks k in [8kb, 8kb+8) of n-tile n
                def wo_batch(w):
                    n, kb = divmod(w, 4)
                    src = wo[1024 * kb:1024 * kb + 1024, 512 * n:512 * n + 512]
                    for s in range(2):
                        ld(
                            S.s_woA if w % 2 == 0 else S.s_woB,
                            wo_sb[:, w % 2, 4 * s:4 * s + 4, :],
                            src[512 * s:512 * s + 512, :].rearrange(
                                "(c p) t -> p c t", p=128
                            ),
                        )

                wo_batch(0)
                wo_batch(1)

                # a2a stores: group g = (h, j), 2 d-chunks x 2 shards
                for g in range(8):
                    h, j = divmod(g, 4)
                    for dc in range(2):
                        sync.wait_ge(S.s_norm, 2 * g + dc + 1)
                        for sh in range(2):
                            sync.dma_start(
                                a2a_in[2 * j + sh,
                                       256 * h + 128 * dc:256 * h + 128 * dc + 128,
                                       :],
                                ot_sb[:, dc, 256 * sh:256 * sh + 256],
                            ).then_inc(S.s_a2a, 16)

                # r load (o_proj lhsT) after the collective
                sync.wait_ge(S.s_cc, 1)
                flat = a2a_out[:, :, :].rearrange("i r t -> (i r) t")
                for s in range(4):
                    ld(
                        S.s_r,
                        r_sb[:, 8 * s:8 * s + 8, :],
                        flat[1024 * s:1024 * s + 1024, :].rearrange(
                            "(c p) t -> p c t", p=128
                        ),
                    )

                # remaining wo batches; buffer w%2 was last used by batch w-2.
                # s_wod incs once per kb<3 batch (3 per n-group, in order);
                # kb==3 batches complete with their n-group (s_po).
                for w in range(2, 24):
                    n2, kb2 = divmod(w - 2, 4)
                    if kb2 == 3:
                        sync.wait_ge(S.s_po, n2 + 1)
                    else:
                        sync.wait_ge(S.s_wod, 3 * n2 + kb2 + 1)
                    wo_batch(w)


            # ---------------- GPSIMD ----------------
            @block.gpsimd
            def _(gp):
                gp.memset(ones_sb[:, :], 1.0)
                gp.memset(ones_row[:, :], 1.0).then_inc(S.s_misc, 1)
                gp.wait_ge(S.s_a2a, 16 * 32)
                if os.environ.get("KERNEL_NO_CC"):
                    gp.dma_start(a2a_out[:, :, :], a2a_in[:, :, :]).then_inc(
                        S.s_cc, 16
                    )
                else:
                    gp.collective_compute(
                        "AllToAll",
                        mybir.AluOpType.bypass,
                        replica_groups=[list(range(N_CORES))],
                        ins=[a2a_in[:, :, :]],
                        outs=[a2a_out[:, :, :]],
                    ).then_inc(S.s_cc, 1)
                # final output stores (gpsimd queue: sync's stream is busy
                # gating wo prefetch on PE progress, which would cycle)
                for n in range(6):
                    for m in range(2):
                        gp.wait_ge(S.s_oc, 2 * n + m + 1)
                        gp.dma_start(
                            out[128 * m:128 * m + 128, 512 * n:512 * n + 512],
                            outst[:, m, :],
                        ).then_inc(S.s_out, 16)

            # ---------------- TENSOR (PE) ----------------
            @block.tensor
            def _(pe):
                # phase 1: qkT + v projections
                for b in range(4):
                    for m in range(6):
                        g = 6 * b + m
                        if g >= 2:
                            # bank g%2 was read by the RoPE pair containing
                            # group g-2; that pair completes at s_pqd = g for
                            # even g and g-1 for odd g
                            pe.wait_ge(S.s_pqd, g if g % 2 == 0 else g - 1)
                        if m == 0:
                            pe.wait_ge([S.s_xt0, S.s_xt1, S.s_xt2, S.s_xt3][b],
                                       16 * 8)
                        for kc in range(KC):
                            ins = pe.matmul(
                                A[g % 2][:, :],
                                lhsT=wqk_sb[:, kc, 128 * m:128 * m + 128],
                                rhs=xt_sb[:, b % 2, kc, :],
                                start=(kc == 0),
                                stop=(kc == KC - 1),
                            )
                        ins.then_inc(S.s_pq, 1)
                    for ts in range(4):
                        vg = 4 * b + ts
                        if vg >= 2:
                            pe.wait_ge(S.s_vcp, vg - 1)
                        for kc in range(KC):
                            ins = pe.matmul(
                                B[vg % 2][:, 0:256],
                                lhsT=xt_sb[:, b % 2, kc, 128 * ts:128 * ts + 128],
                                rhs=wv_sb[:, kc, :],
                                start=(kc == 0),
                                stop=(kc == KC - 1),
                            )
                        ins.then_inc(S.s_pv, 1)

                # phase 2: attention, software-pipelined S^T ahead of consume
                def emit_st(idx):
                    h, j, i = TILES[idx]
                    if idx == 0:
                        pe.wait_ge(S.s_pqd, 24)
                        pe.wait_ge(S.s_vcp, 16)
                    if idx >= 2:
                        pe.wait_ge(S.s_exp, idx - 1)
                    for dc in range(2):
                        ins = pe.matmul(
                            A[idx % 2][:, :],
                            lhsT=qk_sb[:, 4 + dc, 128 * i:128 * i + 128],
                            rhs=qk_sb[:, 2 * h + dc, 512 * j:512 * j + 512],
                            start=(dc == 0),
                            stop=(dc == 1),
                        )
                    ins.then_inc(S.s_stp, 1)

                def emit_cons(idx):
                    h, j, i = TILES[idx]
                    g = GROUP_OF[idx]
                    first = idx == G_FIRST[g]
                    last = idx == G_LAST[g]
                    if i >= 4 * j:
                        pe.wait_ge(S.s_mask, DIAG_COUNT[idx])
                    else:
                        pe.wait_ge(S.s_exp, idx + 1)
                    if first:
                        pe.wait_ge(S.s_rc, g)  # cs bank free (g=0 trivially)
                        if g == 0:
                            pe.wait_ge(S.s_misc, 1)
                        if g >= 1:
                            pe.wait_ge(S.s_norm, 2 * g)  # ot banks free
                    pt = pt_sb[:, idx % 4, :]
                    pe.matmul(
                        CS[0:1, :], lhsT=ones_sb[:, :], rhs=pt,
                        start=first, stop=last,
                    )
                    av = [
                        pe.matmul(
                            A[2 + dc][:, :],
                            lhsT=v_sb[:, i, 128 * dc:128 * dc + 128],
                            rhs=pt,
                            start=first,
                            stop=last,
                        )
                        for dc in range(2)
                    ]
                    # pt-slot free is signalled on av0; av1 (which still reads
                    # pt) is ordered before the slot's next writer because ACT
                    # also waits on s_stp of a later st group (PE in-order).
                    av[0].then_inc(S.s_ptc, 1)
                    if last:
                        av[1].then_inc(S.s_av, 1)
                        # broadcast 1/colsum across partitions via K=1 matmul
                        # (BC bank reuse is guarded by the next group's
                        # s_norm wait; rcrow is ready at s_rc >= g+1)
                        pe.wait_ge(S.s_rc, g + 1)
                        pe.matmul(
                            BC[:, :], lhsT=ones_row[0:1, :], rhs=rcrow[0:1, :],
                            start=True, stop=True,
                        ).then_inc(S.s_bc, 1)

                emit_st(0)
                for idx in range(N_TILES):
                    if idx + 1 < N_TILES:
                        emit_st(idx + 1)
                    emit_cons(idx)

                # phase 3: o_proj
                for n in range(6):
                    for kb in range(4):
                        w = 4 * n + kb
                        if w == 0:
                            pe.wait_ge(S.s_r, 16 * 4)
                        pe.wait_ge(S.s_woA if w % 2 == 0 else S.s_woB,
                                   16 * 2 * (w // 2 + 1))
                        if kb == 0:
                            if n == 0:
                                pe.wait_ge(S.s_exp, N_TILES)
                            elif n == 1:
                                pe.wait_ge(S.s_norm, 16)
                            else:
                                pe.wait_ge(S.s_oc, 2 * n - 2)
                        for kl in range(8):
                            k = 8 * kb + kl
                            for m in range(2):
                                ins = pe.matmul(
                                    A[2 * (n % 2) + m][:, :],
                                    lhsT=r_sb[:, k, 128 * m:128 * m + 128],
                                    rhs=wo_sb[:, w % 2, kl, :],
                                    start=(k == 0),
                                    stop=(k == 31),
                                )
                                if kl == 7 and m == 1:
                                    # kb<3 batches free their wo buffer here;
                                    # kb==3 batches are covered by s_po.
                                    if k == 31:
                                        ins.then_inc(S.s_po, 1)
                                    else:
                                        ins.then_inc(S.s_wod, 1)

            # ---------------- VECTOR (DVE) ----------------
            @block.vector
            def _(ve):
                # phase 1: RoPE + v copies
                dvec = [0]  # same-engine serialization counter for temps

                for b in range(4):
                    tsl = slice(512 * b, 512 * b + 512)
                    for p in range(3):
                        m = 2 * p
                        g0, g1 = 6 * b + m, 6 * b + m + 1
                        ve.wait_ge(S.s_pq, g1 + 1)
                        if b == 0 and p == 0:
                            ve.wait_ge(S.s_init, 16 * 5)
                        q1, q2 = A[g0 % 2][:, :], A[g1 % 2][:, :]

                        def step(fn, *args, inc_pqd=False):
                            if dvec[0]:
                                ve.wait_ge(S.s_dve, dvec[0])
                            ins = fn(*args)
                            ins.then_inc(S.s_pqd, 2) if inc_pqd else                                 ins.then_inc(S.s_dve, 1)
                            if not inc_pqd:
                                dvec[0] += 1

                        step(ve.tensor_mul, tmpA[:, :], q1, cos_sb[:, tsl])
                        step(ve.tensor_mul, tmpB[:, :], q2, sin_sb[:, tsl])
                        step(ve.tensor_sub, qk_sb[:, m, tsl], tmpA[:, :],
                             tmpB[:, :])
                        step(ve.tensor_mul, tmpA[:, :], q2, cos_sb[:, tsl])
                        step(ve.tensor_mul, tmpB[:, :], q1, sin_sb[:, tsl])
                        step(ve.tensor_add, qk_sb[:, m + 1, tsl], tmpA[:, :],
                             tmpB[:, :], inc_pqd=True)
                    for ts in range(4):
                        vg = 4 * b + ts
                        ve.wait_ge(S.s_pv, vg + 1)
                        ve.tensor_copy(v_sb[:, vg, :], B[vg % 2][:, 0:256]).then_inc(
                            S.s_vcp, 1
                        )

                # phase 2
                for g in range(8):
                    h, j = divmod(g, 4)
                    for i in range(4 * j, 4 * j + 4):
                        idx = TILES.index((h, j, i))
                        mm = i - 4 * j
                        ve.wait_ge(S.s_exp, idx + 1)
                        pt = pt_sb[:, idx % 4, :]
                        ve.tensor_mul(pt, pt, mask_sb[:, mm, :]).then_inc(
                            S.s_mask, 1
                        )
                    ve.wait_ge(S.s_av, g + 1)
                    ve.reciprocal(rcrow[0:1, :], CS[0:1, :]).then_inc(S.s_rc, 1)
                    ve.wait_ge(S.s_bc, g + 1)
                    ve.tensor_copy(rb_sb[:, :], BC[:, :]).then_inc(S.s_dve, 1)
                    dvec[0] += 1
                    for dc in range(2):
                        if dc == 0:
                            ve.wait_ge(S.s_dve, dvec[0])
                            if g >= 1:
                                ve.wait_ge(S.s_a2a, 16 * 4 * g)
                        ve.tensor_mul(
                            ot_sb[:, dc, :], A[2 + dc][:, :], rb_sb[:, :]
                        ).then_inc(S.s_norm, 1)

                # phase 3: output copies
                for n in range(6):
                    ve.wait_ge(S.s_po, n + 1)
                    for m in range(2):
                        if n >= 1 and m == 0:
                            # all stores issued so far are those of groups <= n-1
                            ve.wait_ge(S.s_out, 16 * 2 * n)
                        ve.tensor_copy(
                            outst[:, m, :], A[2 * (n % 2) + m][:, :]
                        ).then_inc(S.s_oc, 1)

            # ---------------- SCALAR (ACT): exp ----------------
            @block.scalar
            def _(sc):
                for idx in range(N_TILES):
                    sc.wait_ge(S.s_stp, idx + 1)
                    if idx >= 4:
                        sc.wait_ge(S.s_ptc, idx - 3)
                    sc.activation(
                        pt_sb[:, idx % 4, :],
                        A[idx % 2][:, :],
                        mybir.ActivationFunctionType.Exp,
                        scale=0.0625,
                    ).then_inc(S.s_exp, 1)

    return nc


# ---------------- host side ----------------

NUM_HEADS = 16
NUM_KV_HEADS = 8
HEAD_DIM = 256
ROPE_THETA = 10000.0


def _prep(x, W_qkv, W_o):
    bf = ml_dtypes.bfloat16
    xt = np.ascontiguousarray(x.T).astype(bf)

    pos = np.arange(T, dtype=np.float64)
    inv_freq = 1.0 / ROPE_THETA ** (
        np.arange(0, HEAD_DIM, 2, dtype=np.float64) / HEAD_DIM
    )
    freqs = pos[:, None] * inv_freq[None, :]  # [T, 128]
    cosT = np.ascontiguousarray(np.cos(freqs).T).astype(np.float32)
    sinT = np.ascontiguousarray(np.sin(freqs).T).astype(np.float32)

    p = np.arange(128)[:, None]
    f = np.arange(512)[None, :]
    masks = np.concatenate(
        [(f - p >= 128 * m).astype(np.float32) for m in range(4)], axis=1
    ).astype(bf)

    # per-core weight slices
    in_maps = []
    for c in range(N_CORES):
        q_cols = np.r_[
            HEAD_DIM * c:HEAD_DIM * (c + 1),
            HEAD_DIM * (c + 8):HEAD_DIM * (c + 9),
        ]
        k_cols = np.arange(
            HEAD_DIM * NUM_HEADS + HEAD_DIM * c,
            HEAD_DIM * NUM_HEADS + HEAD_DIM * (c + 1),
        )
        v_cols = np.arange(
            HEAD_DIM * (NUM_HEADS + NUM_KV_HEADS) + HEAD_DIM * c,
            HEAD_DIM * (NUM_HEADS + NUM_KV_HEADS) + HEAD_DIM * (c + 1),
        )
        wqk = np.ascontiguousarray(
            W_qkv[:, np.r_[q_cols, k_cols]]
        ).astype(bf)
        wv = np.ascontiguousarray(W_qkv[:, v_cols]).astype(bf)
        in_maps.append({"wqk": wqk, "wv": wv})

    # permuted W_o rows: R = 512*i + l  ->  W_o row (i + 8*(l>=256))*256 + l%256
    perm = np.empty(4096, dtype=np.int64)
    for i in range(8):
        for l in range(512):
            head = i if l < 256 else i + 8
            perm[512 * i + l] = head * 256 + (l % 256)
    woP = np.ascontiguousarray(W_o[perm, :]).astype(bf)

    for c in range(N_CORES):
        in_maps[c].update(
            xt=xt, cosT=cosT, sinT=sinT, masks=masks, wo=woP
        )
    return in_maps


_CACHE = {}


def kernel(x, W_qkv, W_o):
    trace = bool(int(os.environ.get("KERNEL_TRACE", "0")))
    in_maps = _prep(
        np.asarray(x, np.float32),
        np.asarray(W_qkv, np.float32),
        np.asarray(W_o, np.float32),
    )
    if "nc" not in _CACHE:
        _CACHE["nc"] = build_program()
    nc = _CACHE["nc"]
    res = run_bass_kernel_spmd(
        nc, in_maps, list(range(N_CORES)), trace=trace,
        trace_cores=[0] if trace else None,
    )
    if trace:
        print(f"HW exec time: {res.exec_time_ns} ns")
        _CACHE["last_result"] = res
    full = np.concatenate(
        [res.results[c]["out"] for c in range(N_CORES)], axis=0
    )
    return full.astype(np.float32)


if __name__ == "__main__":
    rng = np.random.default_rng(0)
    x = rng.standard_normal((T, HID), dtype=np.float32)
    Wq = (rng.standard_normal((HID, 8192), dtype=np.float32) * HID ** -0.5)
    Wo = (rng.standard_normal((4096, HID), dtype=np.float32) * 4096 ** -0.5)
    y = kernel(x, Wq, Wo)
    print("ran:", y.shape, y.dtype)
